# revision 1
# baseline (speedup 1.0000x reference)
"""DeepSets ensemble (segment mean-pool + BN MLP) on 8 TRN2 NeuronCores.

Strategy (data-parallel, per sharding hint):
 - events are split 1024/core; each core's points are bin-packed (FFD) into
   512-pt groups of whole events, zero-padded, so the ragged segment-sum
   becomes a block matmul against a host-built 0/1 selector S.
 - per core: phi1 (x [16,512]-tiles bf16 -> PSUM), fused-relu PSUM->SBUF
   copies (alternating DVE/ACT), phi2 with the h1-tile as the stationary
   matmul operand producing h2^T [128pts, 64] tiles, then pooling matmuls
   h2^T x S accumulating z[64, slots] in PSUM; 1/len applied at eviction.
 - MLP sharded over slots; BatchNorm uses two tiny AllReduces of
   (sum, sum-of-squares); mlp_b1/b2 cancel inside BN; empty-slot
   contributions are corrected analytically after the second AllReduce.
 - host scatters x_scalar / gathers y through the slot<->event map.
"""
import sys
import numpy as np
import ml_dtypes
from contextlib import ExitStack

sys.path.insert(0, "/opt/trn_rl_repo")

import concourse.bacc as bacc
import concourse.tile as tile
from concourse import mybir
from concourse import bass_utils

BF16 = mybir.dt.bfloat16
F32 = mybir.dt.float32
AX = mybir.AxisListType
OP = mybir.AluOpType
ACTF = mybir.ActivationFunctionType

N_CORES = 8
C_IN = 16
F = 64
S_SCALAR = 8
M1, M2 = 128, 64
G = 512
SLOTS = 8
EPS = 1e-5


def _plan_groups(lengths, b_total):
    e_per_core = b_total // N_CORES
    cores = []
    for c in range(N_CORES):
        evs = sorted(range(c * e_per_core, (c + 1) * e_per_core),
                     key=lambda e: -int(lengths[e]))
        groups, space = [], []
        for e in evs:
            l = int(lengths[e])
            assert 0 < l <= G
            placed = False
            for gi in range(len(groups)):
                if space[gi] >= l and len(groups[gi]) < SLOTS:
                    groups[gi].append(e)
                    space[gi] -= l
                    placed = True
                    break
            if not placed:
                groups.append([e])
                space.append(G - l)
        cores.append(groups)
    ng = max(len(g) for g in cores)
    if ng % 2:
        ng += 1
    for g in cores:
        while len(g) < ng:
            g.append([])
    return cores, ng


def _prep_core(x, x_scalar, lengths, offsets, groups, ng):
    nchunk = ng // 2
    p_pad = ng * G
    sl = ng * SLOTS
    nkt = p_pad // 128
    xb = np.zeros((2 * C_IN, p_pad // 2), dtype=np.float32)
    s_mat = np.zeros((128, nkt * SLOTS), dtype=np.float32)
    recip = np.zeros((1, sl), dtype=np.float32)
    xsT = np.zeros((S_SCALAR, sl), dtype=np.float32)
    slot_events = np.full(sl, -1, dtype=np.int64)
    for j in range(nchunk):
        for half, g_idx in ((0, j), (1, nchunk + j)):
            evs = groups[g_idx]
            col0 = 512 * j
            row0 = C_IN * half
            kt0 = 8 * j + 4 * half
            slot0 = 8 * g_idx
            pt = 0
            for i, e in enumerate(evs):
                l = int(lengths[e])
                o = int(offsets[e])
                xb[row0:row0 + C_IN, col0 + pt: col0 + pt + l] = x[:, o:o + l]
                p_arr = np.arange(pt, pt + l)
                s_mat[p_arr % 128, (kt0 + p_arr // 128) * SLOTS + i] = 1.0
                slot = slot0 + i
                recip[0, slot] = 1.0 / l
                xsT[:, slot] = x_scalar[e]
                slot_events[slot] = e
                pt += l
    return {
        "xb": np.ascontiguousarray(xb.astype(ml_dtypes.bfloat16)),
        "S": np.ascontiguousarray(s_mat.astype(ml_dtypes.bfloat16)),
        "recip": recip,
        "xsT": xsT,
        "slot_events": slot_events,
    }


def _build_nc(ng, sl, b_total):
    nchunk = ng // 2
    p_pad = ng * G
    nkt = p_pad // 128
    n_empty = float(N_CORES * sl - b_total)
    inv_b = 1.0 / float(b_total)

    nc = bacc.Bacc("TRN2", target_bir_lowering=False, debug=False,
                   num_devices=N_CORES)

    def din(name, shape, dt):
        return nc.dram_tensor(name, shape, dt, kind="ExternalInput").ap()

    xb = din("xb", [2 * C_IN, p_pad // 2], BF16)
    w1s = din("w1s", [2 * C_IN, 128], BF16)
    w2s = din("w2s", [128, 128], BF16)
    s_in = din("S", [128, nkt * SLOTS], BF16)
    recip = din("recip", [1, sl], F32)
    xsT = din("xsT", [S_SCALAR, sl], F32)
    w1t = din("w1t", [C_IN, F], BF16)
    w2t = din("w2t", [F, F], BF16)
    mw1t = din("mw1t", [F + S_SCALAR, M1], F32)
    mw2t = din("mw2t", [M1, M2], F32)
    mw3t = din("mw3t", [M2, 1], F32)
    bn1_g = din("bn1_g", [M1, 1], F32)
    bn1_b = din("bn1_b", [M1, 1], F32)
    bn2_g = din("bn2_g", [M2, 1], F32)
    bn2_b = din("bn2_b", [M2, 1], F32)
    b3 = din("b3", [1, 1], F32)

    y_out = nc.dram_tensor("y", [1, sl], F32, kind="ExternalOutput").ap()

    XCHUNK = 16
    SCHUNK = 16
    n512 = (sl + 511) // 512

    def col_tiles():
        for i in range(n512):
            yield i * 512, min(sl, (i + 1) * 512)

    with tile.TileContext(nc) as tc, ExitStack() as ctx:
        const_pool = ctx.enter_context(tc.tile_pool(name="const", bufs=1))
        xb_pool = ctx.enter_context(tc.tile_pool(name="xb", bufs=2))
        s_pool = ctx.enter_context(tc.tile_pool(name="spool", bufs=2))
        h1_pool = ctx.enter_context(tc.tile_pool(name="h1", bufs=3))
        h2_pool = ctx.enter_context(tc.tile_pool(name="h2", bufs=3))
        z_pool = ctx.enter_context(tc.tile_pool(name="z", bufs=1))
        mlp_pool = ctx.enter_context(tc.tile_pool(name="mlp", bufs=1))
        stat_pool = ctx.enter_context(tc.tile_pool(name="stat", bufs=1))
        ps_a = ctx.enter_context(tc.tile_pool(name="psa", bufs=2, space="PSUM"))
        ps_b = ctx.enter_context(tc.tile_pool(name="psb", bufs=2, space="PSUM"))
        ps_z = ctx.enter_context(tc.tile_pool(name="psz", bufs=2, space="PSUM"))
        dram = ctx.enter_context(tc.tile_pool(name="dram", bufs=1, space="DRAM"))

        w1s_s = const_pool.tile([2 * C_IN, 128], BF16)
        nc.sync.dma_start(w1s_s[:], w1s[:])
        # phi2 runs full-K (128) matmuls over the stacked [h1A; h1B] tile with
        # zero-padded weight halves selecting the A or B rows; every matmul
        # operand stays at partition base 0 (PE hangs when stationary-operand
        # loads alternate between base 0 and base 64).
        w2s_s = const_pool.tile([128, 128], BF16)
        nc.sync.dma_start(w2s_s[:], w2s[:])
        mw1t_s = const_pool.tile([F + S_SCALAR, M1], F32)
        nc.sync.dma_start(mw1t_s[:], mw1t[:])
        mw2t_s = const_pool.tile([M1, M2], F32)
        nc.sync.dma_start(mw2t_s[:], mw2t[:])
        mw3t_s = const_pool.tile([M2, 1], F32)
        nc.sync.dma_start(mw3t_s[:], mw3t[:])
        g1_s = const_pool.tile([M1, 1], F32); nc.sync.dma_start(g1_s[:], bn1_g[:])
        bb1_s = const_pool.tile([M1, 1], F32); nc.sync.dma_start(bb1_s[:], bn1_b[:])
        g2_s = const_pool.tile([M2, 1], F32); nc.sync.dma_start(g2_s[:], bn2_g[:])
        bb2_s = const_pool.tile([M2, 1], F32); nc.sync.dma_start(bb2_s[:], bn2_b[:])
        b3_s = const_pool.tile([1, 1], F32); nc.sync.dma_start(b3_s[:], b3[:])
        recip_s = const_pool.tile([1, sl], F32)
        nc.sync.dma_start(recip_s[:], recip[:])

        z_s = z_pool.tile([F + S_SCALAR, sl], F32)
        nc.sync.dma_start(z_s[F:F + S_SCALAR, :], xsT[:])

        ones1 = const_pool.tile([1, F], F32)
        nc.vector.memset(ones1[:], 1.0)
        r_s = const_pool.tile([F, sl], F32)
        for a, b in col_tiles():
            rp = ps_z.tile([128, 512], F32, tag="zpa")
            nc.tensor.matmul(rp[0:F, :b - a], ones1[:], recip_s[:, a:b],
                             start=True, stop=True)
            nc.vector.tensor_copy(r_s[:, a:b], rp[0:F, :b - a])

        zp = None
        for j in range(nchunk):
            if j % XCHUNK == 0:
                nx = min(XCHUNK, nchunk - j)
                xb_t = xb_pool.tile([2 * C_IN, XCHUNK * 512], BF16, tag="xb")
                nc.sync.dma_start(xb_t[:, :nx * 512],
                                  xb[:, j * 512:(j + nx) * 512])
            if j % SCHUNK == 0:
                ns = min(SCHUNK, nchunk - j)
                s_t = s_pool.tile([128, SCHUNK * 8 * SLOTS], BF16, tag="st")
                nc.sync.dma_start(
                    s_t[:, :ns * 8 * SLOTS],
                    s_in[:, j * 8 * SLOTS:(j + ns) * 8 * SLOTS])
            jx = (j % XCHUNK) * 512
            js = (j % SCHUNK) * 8 * SLOTS

            p1 = ps_a.tile([128, 512], F32, tag="p1")
            nc.tensor.matmul(p1[:, :], w1s_s[:], xb_t[:, jx:jx + 512],
                             start=True, stop=True)
            h1_t = h1_pool.tile([128, 512], BF16, tag="h1")
            if j % 2 == 0:
                nc.vector.tensor_scalar(h1_t[:], p1[:], 0.0, None, OP.max)
            else:
                nc.scalar.activation(h1_t[:], p1[:], ACTF.Relu)

            # block-diagonal W2 computes the A-half (cols 0:64) and B-half
            # (cols 64:128) h2^T of each k-tile in one matmul
            p2 = ps_b.tile([128, 512], F32, tag="p2")
            for t in range(4):
                nc.tensor.matmul(p2[:, 128 * t:128 * t + 128],
                                 h1_t[:, 128 * t:128 * t + 128], w2s_s[:],
                                 start=True, stop=True)
            h2_t = h2_pool.tile([128, 512], BF16, tag="h2")
            if j % 2 == 1:
                nc.vector.tensor_scalar(h2_t[:], p2[:], 0.0, None, OP.max)
            else:
                nc.scalar.activation(h2_t[:], p2[:], ACTF.Relu)

            # pool: one [128,128] stationary load serves the A and B matmul
            # of each k-tile pair; A-partials at psum rows 0:64 (bank zpa),
            # B-partials at rows 64:128 (bank zpb); garbage halves of each
            # product land in rows that are never evicted.
            if j % 64 == 0:
                zpa = ps_z.tile([128, 512], F32, tag="zpa")
                zpb = ps_z.tile([128, 512], F32, tag="zpb")
            zc = (j % 64) * 8
            for t in range(4):
                lhs = h2_t[:, 128 * t:128 * t + 128]
                nc.tensor.matmul(
                    zpa[:, zc:zc + SLOTS], lhs,
                    s_t[:, js + t * SLOTS: js + (t + 1) * SLOTS],
                    start=(t == 0), stop=(t == 3))
                nc.tensor.matmul(
                    zpb[:, zc:zc + SLOTS], lhs,
                    s_t[:, js + (4 + t) * SLOTS: js + (5 + t) * SLOTS],
                    start=(t == 0), stop=(t == 3))
            if (j + 1) % 64 == 0 or j == nchunk - 1:
                a = (j // 64) * 512
                w = (j % 64) * 8 + 8
                hb = sl // 2
                nc.vector.tensor_tensor(z_s[0:F, a:a + w], zpa[0:F, :w],
                                        r_s[:, a:a + w], OP.mult)
                nc.vector.tensor_tensor(z_s[0:F, hb + a:hb + a + w],
                                        zpb[F:2 * F, :w],
                                        r_s[:, hb + a:hb + a + w], OP.mult)

        # ---------- MLP ----------
        v1_s = mlp_pool.tile([M1, sl], F32, tag="v1")
        for a, b in col_tiles():
            pv = ps_a.tile([M1, 512], F32, tag="p1")
            nc.tensor.matmul(pv[:, :b - a], mw1t_s[:], z_s[:, a:b],
                             start=True, stop=True)
            nc.vector.tensor_copy(v1_s[:, a:b], pv[:, :b - a])

        s1 = stat_pool.tile([M1, 2], F32, tag="s1")
        nc.vector.tensor_reduce(s1[:, 0:1], v1_s[:], axis=AX.X, op=OP.add)
        sq_scr = mlp_pool.tile([M1, sl], F32, tag="scr")
        nc.scalar.activation(sq_scr[:], v1_s[:], ACTF.Square,
                             accum_out=s1[:, 1:2])
        cc_in1 = dram.tile([M1, 2], F32)
        cc_out1 = dram.tile([M1, 2], F32)
        nc.sync.dma_start(cc_in1[:], s1[:])
        nc.gpsimd.collective_compute(
            "AllReduce", OP.add, replica_groups=[list(range(N_CORES))],
            ins=[cc_in1.opt()], outs=[cc_out1.opt()])
        ar1 = stat_pool.tile([M1, 2], F32, tag="ar1")
        nc.sync.dma_start(ar1[:], cc_out1[:])

        t1 = stat_pool.tile([M1, 6], F32, tag="t1")
        mean1, var1, inv1, sd1, sc1, of1 = (t1[:, i:i + 1] for i in range(6))
        nc.scalar.mul(mean1, ar1[:, 0:1], inv_b)
        nc.vector.tensor_tensor(var1, mean1, mean1, OP.mult)
        nc.vector.tensor_scalar(var1, var1, -1.0, None, OP.mult)
        nc.vector.scalar_tensor_tensor(
            var1, ar1[:, 1:2], inv_b, var1, OP.mult, OP.add)
        nc.vector.tensor_scalar(var1, var1, EPS, None, OP.add)
        nc.vector.reciprocal(inv1, var1)
        nc.scalar.activation(sd1, inv1, ACTF.Sqrt)
        nc.vector.tensor_tensor(sc1, g1_s[:], sd1, OP.mult)
        nc.vector.tensor_tensor(of1, mean1, sc1, OP.mult)
        nc.vector.tensor_tensor(of1, bb1_s[:], of1, OP.subtract)

        a1_s = mlp_pool.tile([M1, sl], F32, tag="v1a")
        nc.scalar.activation(a1_s[:], v1_s[:], ACTF.Relu, bias=of1, scale=sc1)

        v2_s = mlp_pool.tile([M2, sl], F32, tag="v2")
        for a, b in col_tiles():
            pv = ps_b.tile([M2, 512], F32, tag="p2")
            nc.tensor.matmul(pv[:, :b - a], mw2t_s[:], a1_s[:, a:b],
                             start=True, stop=True)
            nc.vector.tensor_copy(v2_s[:, a:b], pv[:, :b - a])

        s2 = stat_pool.tile([M2, 2], F32, tag="s2")
        nc.vector.tensor_reduce(s2[:, 0:1], v2_s[:], axis=AX.X, op=OP.add)
        sq2_scr = mlp_pool.tile([M2, sl], F32, tag="scr2")
        nc.scalar.activation(sq2_scr[:], v2_s[:], ACTF.Square,
                             accum_out=s2[:, 1:2])
        cc_in2 = dram.tile([M2, 2], F32)
        cc_out2 = dram.tile([M2, 2], F32)
        nc.sync.dma_start(cc_in2[:], s2[:])
        nc.gpsimd.collective_compute(
            "AllReduce", OP.add, replica_groups=[list(range(N_CORES))],
            ins=[cc_in2.opt()], outs=[cc_out2.opt()])
        ar2 = stat_pool.tile([M2, 2], F32, tag="ar2")
        nc.sync.dma_start(ar2[:], cc_out2[:])

        # empty-slot correction: a1_empty = relu(C1); ve = W2 @ a1_empty
        a1e = stat_pool.tile([M1, 1], F32, tag="a1e")
        nc.scalar.activation(a1e[:], of1, ACTF.Relu)
        pve = ps_b.tile([M2, 512], F32, tag="p2")
        nc.tensor.matmul(pve[:, 0:1], mw2t_s[:], a1e[:], start=True, stop=True)
        ve = stat_pool.tile([M2, 3], F32, tag="ve")
        nc.vector.tensor_copy(ve[:, 0:1], pve[:, 0:1])
        nc.vector.tensor_tensor(ve[:, 1:2], ve[:, 0:1], ve[:, 0:1], OP.mult)
        s2c = stat_pool.tile([M2, 2], F32, tag="s2c")
        nc.vector.scalar_tensor_tensor(
            s2c[:, 0:1], ve[:, 0:1], -n_empty, ar2[:, 0:1], OP.mult, OP.add)
        nc.vector.scalar_tensor_tensor(
            s2c[:, 1:2], ve[:, 1:2], -n_empty, ar2[:, 1:2], OP.mult, OP.add)

        t2 = stat_pool.tile([M2, 6], F32, tag="t2")
        mean2, var2, inv2, sd2, sc2, of2 = (t2[:, i:i + 1] for i in range(6))
        nc.scalar.mul(mean2, s2c[:, 0:1], inv_b)
        nc.vector.tensor_tensor(var2, mean2, mean2, OP.mult)
        nc.vector.tensor_scalar(var2, var2, -1.0, None, OP.mult)
        nc.vector.scalar_tensor_tensor(
            var2, s2c[:, 1:2], inv_b, var2, OP.mult, OP.add)
        nc.vector.tensor_scalar(var2, var2, EPS, None, OP.add)
        nc.vector.reciprocal(inv2, var2)
        nc.scalar.activation(sd2, inv2, ACTF.Sqrt)
        nc.vector.tensor_tensor(sc2, g2_s[:], sd2, OP.mult)
        nc.vector.tensor_tensor(of2, mean2, sc2, OP.mult)
        nc.vector.tensor_tensor(of2, bb2_s[:], of2, OP.subtract)

        a2_s = mlp_pool.tile([M2, sl], F32, tag="v2a")
        nc.scalar.activation(a2_s[:], v2_s[:], ACTF.Relu, bias=of2, scale=sc2)

        y_s = mlp_pool.tile([1, sl], F32, tag="y")
        for a, b in col_tiles():
            pv = ps_a.tile([M1, 512], F32, tag="p1")
            nc.tensor.matmul(pv[0:1, :b - a], mw3t_s[:], a2_s[:, a:b],
                             start=True, stop=True)
            nc.scalar.activation(y_s[:, a:b], pv[0:1, :b - a], ACTF.Sigmoid,
                                 bias=b3_s[:])
        nc.sync.dma_start(y_out[:], y_s[:])

    nc.compile()
    return nc


_CACHE = {}


def kernel(**inputs) -> np.ndarray:
    x = np.asarray(inputs["x_set"], np.float32)[0]        # [16, T]
    x_scalar = np.asarray(inputs["x_scalar"], np.float32)  # [B, 8]
    lengths = np.asarray(inputs["sample_indices"])[0].astype(np.int64)
    b_total = x_scalar.shape[0]
    offsets = np.concatenate([[0], np.cumsum(lengths)[:-1]])

    groups, ng = _plan_groups(lengths, b_total)
    sl = ng * SLOTS
    per_core = [
        _prep_core(x, x_scalar, lengths, offsets, groups[c], ng)
        for c in range(N_CORES)
    ]

    w1t = np.ascontiguousarray(
        np.asarray(inputs["phi_w1"], np.float32).T.astype(ml_dtypes.bfloat16))
    w2t = np.ascontiguousarray(
        np.asarray(inputs["phi_w2"], np.float32).T.astype(ml_dtypes.bfloat16))
    b1 = np.asarray(inputs["phi_b1"], np.float32)
    b2 = np.asarray(inputs["phi_b2"], np.float32)
    assert np.all(b1 == 0.0) and np.all(b2 == 0.0), \
        "nonzero phi bias path not implemented"
    w1s = np.zeros((2 * C_IN, 128), dtype=np.float32)
    w1s[0:C_IN, 0:F] = np.asarray(inputs["phi_w1"], np.float32).T
    w1s[C_IN:2 * C_IN, F:128] = np.asarray(inputs["phi_w1"], np.float32).T
    w2s = np.zeros((128, 128), dtype=np.float32)
    w2s[0:F, 0:F] = np.asarray(inputs["phi_w2"], np.float32).T
    w2s[F:128, F:128] = np.asarray(inputs["phi_w2"], np.float32).T
    consts = {
        "w2s": np.ascontiguousarray(w2s.astype(ml_dtypes.bfloat16)),
        "w1s": np.ascontiguousarray(w1s.astype(ml_dtypes.bfloat16)),
        "w1t": w1t,
        "w2t": w2t,
        "mw1t": np.ascontiguousarray(np.asarray(inputs["mlp_w1"], np.float32).T),
        "mw2t": np.ascontiguousarray(np.asarray(inputs["mlp_w2"], np.float32).T),
        "mw3t": np.ascontiguousarray(np.asarray(inputs["mlp_w3"], np.float32).T),
        "bn1_g": np.asarray(inputs["bn1_g"], np.float32).reshape(M1, 1),
        "bn1_b": np.asarray(inputs["bn1_b"], np.float32).reshape(M1, 1),
        "bn2_g": np.asarray(inputs["bn2_g"], np.float32).reshape(M2, 1),
        "bn2_b": np.asarray(inputs["bn2_b"], np.float32).reshape(M2, 1),
        "b3": np.asarray(inputs["mlp_b3"], np.float32).reshape(1, 1),
    }

    key = (ng, sl, b_total)
    if key not in _CACHE:
        _CACHE[key] = _build_nc(ng, sl, b_total)
    nc = _CACHE[key]

    in_maps = []
    for pc in per_core:
        m = {"xb": pc["xb"], "S": pc["S"], "recip": pc["recip"],
             "xsT": pc["xsT"]}
        m.update(consts)
        in_maps.append(m)

    res = bass_utils.run_bass_kernel_spmd(
        nc, in_maps, core_ids=list(range(N_CORES)))

    y = np.zeros((b_total, 1), dtype=np.float32)
    for c, pc in enumerate(per_core):
        ys = res.results[c]["y"][0]
        se = pc["slot_events"]
        mask = se >= 0
        y[se[mask], 0] = ys[mask]
    return y



# revision 9
# speedup vs baseline: 1.1024x; 1.1024x over previous
"""DeepSets ensemble (segment mean-pool + BN MLP) on 8 TRN2 NeuronCores.

Strategy (data-parallel, per sharding hint):
 - events are split 1024/core; each core's points are bin-packed (FFD) into
   512-pt groups of whole events (<=6 per group), zero-padded, so the ragged
   segment-sum becomes a block matmul against a host-built selector S whose
   entries are 1/len (mean pooling folded into the matmul).
 - per core: phi1 (x [16,512]-tiles bf16 -> PSUM), fused-relu PSUM->SBUF
   copies (alternating DVE/ACT), phi2 with the h1-tile as the stationary
   matmul operand producing h2^T [128pts, 64] tiles, then pooling matmuls
   h2^T x S (12 merged A/B slot cols per k-tile) accumulating z in PSUM.
 - MLP sharded over slots, computed in bf16; BatchNorm uses two tiny
   AllReduces of (sum, sum-of-squares); a dummy AllReduce at kernel start
   warms the collective stream; v1 + its stats are pipelined into the main
   loop per flushed PSUM bank; empty-slot contributions are corrected
   analytically after the second AllReduce.
 - host scatters x_scalar / gathers y through the slot<->event map.
"""
import sys
import numpy as np
import ml_dtypes
from contextlib import ExitStack

sys.path.insert(0, "/opt/trn_rl_repo")

import concourse.bacc as bacc
import concourse.tile as tile
from concourse import mybir
from concourse import bass_utils

BF16 = mybir.dt.bfloat16
F32 = mybir.dt.float32
AX = mybir.AxisListType
OP = mybir.AluOpType
ACTF = mybir.ActivationFunctionType

N_CORES = 8
C_IN = 16
F = 64
S_SCALAR = 8
M1, M2 = 128, 64
G = 512
SLOTS = 6
NB = 42          # chunks per pooling PSUM bank (12 cols each -> 504)
EPS = 1e-5


def _plan_groups(lengths, b_total):
    e_per_core = b_total // N_CORES
    cores = []
    for c in range(N_CORES):
        evs = sorted(range(c * e_per_core, (c + 1) * e_per_core),
                     key=lambda e: -int(lengths[e]))
        groups, space = [], []
        for e in evs:
            l = int(lengths[e])
            assert 0 < l <= G
            placed = False
            for gi in range(len(groups)):
                if space[gi] >= l and len(groups[gi]) < SLOTS:
                    groups[gi].append(e)
                    space[gi] -= l
                    placed = True
                    break
            if not placed:
                groups.append([e])
                space.append(G - l)
        cores.append(groups)
    ng = max(len(g) for g in cores)
    if ng % 2:
        ng += 1
    for g in cores:
        while len(g) < ng:
            g.append([])
    return cores, ng


def _prep_core(x, x_scalar, lengths, offsets, groups, ng):
    nchunk = ng // 2
    p_pad = ng * G
    sl = ng * SLOTS
    xb = np.zeros((2 * C_IN, p_pad // 2), dtype=np.float32)
    # merged A/B selector: per chunk j, 4 k-tile blocks of 12 cols
    # (6 A-slot cols then 6 B-slot cols), entries are 1/len
    s_mat = np.zeros((128, nchunk * 4 * 12), dtype=np.float32)
    xsT = np.zeros((S_SCALAR, sl), dtype=np.float32)
    slot_events = np.full(sl, -1, dtype=np.int64)
    for j in range(nchunk):
        for half, g_idx in ((0, j), (1, nchunk + j)):
            evs = groups[g_idx]
            col0 = 512 * j
            row0 = C_IN * half
            pt = 0
            for i, e in enumerate(evs):
                l = int(lengths[e])
                o = int(offsets[e])
                xb[row0:row0 + C_IN, col0 + pt: col0 + pt + l] = x[:, o:o + l]
                p_arr = np.arange(pt, pt + l)
                s_mat[p_arr % 128,
                      (4 * j + p_arr // 128) * 12 + 6 * half + i] = 1.0 / l
                slot = SLOTS * g_idx + i
                xsT[:, slot] = x_scalar[e]
                slot_events[slot] = e
                pt += l
    return {
        "xb": np.ascontiguousarray(xb.astype(ml_dtypes.bfloat16)),
        "S": np.ascontiguousarray(s_mat.astype(ml_dtypes.bfloat16)),
        "xsT": np.ascontiguousarray(xsT.astype(ml_dtypes.bfloat16)),
        "slot_events": slot_events,
    }


def _build_nc(ng, sl, b_total):
    nchunk = ng // 2
    p_pad = ng * G
    n_empty = float(N_CORES * sl - b_total)
    inv_b = 1.0 / float(b_total)
    hb = sl // 2

    # flush blocks: bank k covers chunks [k*NB, min((k+1)*NB, nchunk))
    nflush = (nchunk + NB - 1) // NB
    nreg = 2 * nflush

    nc = bacc.Bacc("TRN2", target_bir_lowering=False, debug=False,
                   num_devices=N_CORES)

    def din(name, shape, dt):
        return nc.dram_tensor(name, shape, dt, kind="ExternalInput").ap()

    xb = din("xb", [2 * C_IN, p_pad // 2], BF16)
    s_in = din("S", [128, nchunk * 4 * 12], BF16)
    xsT = din("xsT", [S_SCALAR, sl], BF16)
    # bf16 const blob: w2s cols 0:128, w1s cols 128:256 (rows 0:32),
    # mw1t cols 256:384 (rows 0:72), mw2t cols 384:448, mw3t col 448
    cb = din("cb", [128, 449], BF16)
    # f32 const blob: bn1_g, bn1_b, bn2_g, bn2_b, b3 (col 4, row 0)
    cf = din("cf", [128, 5], F32)

    y_out = nc.dram_tensor("y", [1, sl], F32, kind="ExternalOutput").ap()

    XCHUNK = 16
    SCHUNK = 16

    def col_tiles():
        n512 = (sl + 511) // 512
        for i in range(n512):
            yield i * 512, min(sl, (i + 1) * 512)

    with tile.TileContext(nc) as tc, ExitStack() as ctx:
        const_pool = ctx.enter_context(tc.tile_pool(name="const", bufs=1))
        xb_pool = ctx.enter_context(tc.tile_pool(name="xb", bufs=2))
        s_pool = ctx.enter_context(tc.tile_pool(name="spool", bufs=2))
        h1_pool = ctx.enter_context(tc.tile_pool(name="h1", bufs=3))
        h2_pool = ctx.enter_context(tc.tile_pool(name="h2", bufs=3))
        z_pool = ctx.enter_context(tc.tile_pool(name="z", bufs=1))
        mlp_pool = ctx.enter_context(tc.tile_pool(name="mlp", bufs=1))
        stat_pool = ctx.enter_context(tc.tile_pool(name="stat", bufs=1))
        ps_a = ctx.enter_context(tc.tile_pool(name="psa", bufs=2, space="PSUM"))
        ps_b = ctx.enter_context(tc.tile_pool(name="psb", bufs=2, space="PSUM"))
        ps_z = ctx.enter_context(tc.tile_pool(name="psz", bufs=2, space="PSUM"))
        ps_m = ctx.enter_context(tc.tile_pool(name="psm", bufs=2, space="PSUM"))
        dram = ctx.enter_context(tc.tile_pool(name="dram", bufs=1, space="DRAM"))

        # ---------- startup: data DMAs first, then consts, then CC warmup ----
        xb_t = xb_pool.tile([2 * C_IN, XCHUNK * 512], BF16, tag="xb")
        nc.sync.dma_start(xb_t[:], xb[:, 0:XCHUNK * 512])
        s_t = s_pool.tile([128, SCHUNK * 48], BF16, tag="st")
        nc.sync.dma_start(s_t[:], s_in[:, 0:SCHUNK * 48])

        cb_s = const_pool.tile([128, 449], BF16)
        nc.scalar.dma_start(cb_s[:], cb[:])
        cf_s = const_pool.tile([128, 5], F32)
        nc.scalar.dma_start(cf_s[:], cf[:])

        z_s = z_pool.tile([F + S_SCALAR, sl], BF16)
        nc.scalar.dma_start(z_s[F:F + S_SCALAR, :], xsT[:])

        w2s_s = cb_s[:, 0:128]
        w1s_s = cb_s[0:2 * C_IN, 128:256]
        mw1t_s = cb_s[0:F + S_SCALAR, 256:384]
        mw2t_s = cb_s[:, 384:448]
        mw3t_s = cb_s[0:M2, 448:449]
        g1_s = cf_s[:, 0:1]
        bb1_s = cf_s[:, 1:2]
        g2_s = cf_s[0:M2, 2:3]
        bb2_s = cf_s[0:M2, 3:4]
        b3_s = cf_s[0:1, 4:5]

        # warm up the collective stream so the real AllReduces are cheap
        warm_in = dram.tile([1, 1], F32)
        warm_out = dram.tile([1, 1], F32)
        wsrc = const_pool.tile([1, 1], F32)
        nc.vector.memset(wsrc[:], 0.0)
        nc.gpsimd.dma_start(warm_in[:], wsrc[:])
        nc.gpsimd.collective_compute(
            "AllReduce", OP.add, replica_groups=[list(range(N_CORES))],
            ins=[warm_in.opt()], outs=[warm_out.opt()])

        # v1 partial stats: per flush-region sums / sum-of-squares
        sp_sum = stat_pool.tile([M1, nreg], F32, tag="spsum")
        sp_sq = stat_pool.tile([M1, nreg], F32, tag="spsq")
        v1_s = mlp_pool.tile([M1, sl], F32, tag="v1")

        # ---------- main loop ----------
        zp = None
        for j in range(nchunk):
            if j % XCHUNK == 0 and j > 0:
                nx = min(XCHUNK, nchunk - j)
                xb_t = xb_pool.tile([2 * C_IN, XCHUNK * 512], BF16, tag="xb")
                nc.sync.dma_start(xb_t[:, :nx * 512],
                                  xb[:, j * 512:(j + nx) * 512])
            if j % SCHUNK == 0 and j > 0:
                ns = min(SCHUNK, nchunk - j)
                s_t = s_pool.tile([128, SCHUNK * 48], BF16, tag="st")
                nc.sync.dma_start(s_t[:, :ns * 48],
                                  s_in[:, j * 48:(j + ns) * 48])
            jx = (j % XCHUNK) * 512
            js = (j % SCHUNK) * 48

            p1 = ps_a.tile([128, 512], F32, tag="p1")
            nc.tensor.matmul(p1[:, :], w1s_s, xb_t[:, jx:jx + 512],
                             start=True, stop=True)
            h1_t = h1_pool.tile([128, 512], BF16, tag="h1")
            if j % 2 == 0:
                nc.vector.tensor_scalar(h1_t[:], p1[:], 0.0, None, OP.max)
            else:
                nc.scalar.activation(h1_t[:], p1[:], ACTF.Relu)

            # block-diagonal W2 computes the A-half (cols 0:64) and B-half
            # (cols 64:128) h2^T of each k-tile in one matmul
            p2 = ps_b.tile([128, 512], F32, tag="p2")
            for t in range(4):
                nc.tensor.matmul(p2[:, 128 * t:128 * t + 128],
                                 h1_t[:, 128 * t:128 * t + 128], w2s_s,
                                 start=True, stop=True)
            h2_t = h2_pool.tile([128, 512], BF16, tag="h2")
            if j % 2 == 1:
                nc.vector.tensor_scalar(h2_t[:], p2[:], 0.0, None, OP.max)
            else:
                nc.scalar.activation(h2_t[:], p2[:], ACTF.Relu)

            # pool: one matmul per k-tile; 12 S cols = 6 A-slots (valid at
            # PSUM rows 0:64) + 6 B-slots (valid rows 64:128); garbage
            # quadrants are never evicted.
            if j % NB == 0:
                zp = ps_z.tile([128, 504], F32, tag="zp")
            zc = (j % NB) * 12
            for t in range(4):
                nc.tensor.matmul(
                    zp[:, zc:zc + 12], h2_t[:, 128 * t:128 * t + 128],
                    s_t[:, js + 12 * t: js + 12 * t + 12],
                    start=(t == 0), stop=(t == 3))

            if (j + 1) % NB == 0 or j == nchunk - 1:
                k = j // NB
                j0 = k * NB
                nb = j - j0 + 1
                zv = zp.rearrange("p (b c) -> p b c", c=12)
                for h, (r0, coff) in enumerate(((0, 0), (64, hb))):
                    c0 = coff + 6 * j0
                    dst = z_s[0:F, c0:c0 + 6 * nb].rearrange(
                        "p (b c) -> p b c", c=6)
                    nc.vector.tensor_copy(
                        dst, zv[r0:r0 + F, 0:nb, 6 * h:6 * h + 6])
                    # pipelined v1 + stats for this flushed region
                    pv = ps_m.tile([M1, 504], F32, tag="pm")
                    nc.tensor.matmul(pv[:, :6 * nb], mw1t_s,
                                     z_s[0:F + S_SCALAR, c0:c0 + 6 * nb],
                                     start=True, stop=True)
                    ridx = 2 * k + h
                    nc.scalar.activation(v1_s[:, c0:c0 + 6 * nb],
                                         pv[:, :6 * nb], ACTF.Copy,
                                         accum_out=sp_sum[:, ridx:ridx + 1])
                    scr = mlp_pool.tile([M1, 504], F32, tag="scr", bufs=2)
                    nc.scalar.activation(scr[:, :6 * nb], pv[:, :6 * nb],
                                         ACTF.Square,
                                         accum_out=sp_sq[:, ridx:ridx + 1])

        # ---------- BN1 ----------
        s1 = stat_pool.tile([M1, 2], F32, tag="s1")
        nc.vector.tensor_reduce(s1[:, 0:1], sp_sum[:], axis=AX.X, op=OP.add)
        nc.vector.tensor_reduce(s1[:, 1:2], sp_sq[:], axis=AX.X, op=OP.add)
        cc_in1 = dram.tile([M1, 2], F32)
        cc_out1 = dram.tile([M1, 2], F32)
        nc.sync.dma_start(cc_in1[:], s1[:])
        nc.gpsimd.collective_compute(
            "AllReduce", OP.add, replica_groups=[list(range(N_CORES))],
            ins=[cc_in1.opt()], outs=[cc_out1.opt()])
        ar1 = stat_pool.tile([M1, 2], F32, tag="ar1")
        nc.sync.dma_start(ar1[:], cc_out1[:])

        t1 = stat_pool.tile([M1, 8], F32, tag="t1")
        mean1, m21, var1, inv1, rstd1, sc1, ms1, of1 = (
            t1[:, i:i + 1] for i in range(8))
        nc.vector.tensor_scalar(mean1, ar1[:, 0:1], inv_b, None, OP.mult)
        nc.vector.tensor_tensor(m21, mean1, mean1, OP.mult)
        nc.vector.tensor_scalar(m21, m21, EPS, None, OP.subtract)
        nc.vector.scalar_tensor_tensor(
            var1, ar1[:, 1:2], inv_b, m21, OP.mult, OP.subtract)
        nc.vector.reciprocal(inv1, var1)
        nc.scalar.activation(rstd1, inv1, ACTF.Sqrt)
        nc.vector.tensor_tensor(sc1, g1_s, rstd1, OP.mult)
        nc.vector.tensor_tensor(ms1, mean1, sc1, OP.mult)
        nc.vector.tensor_tensor(of1, bb1_s, ms1, OP.subtract)

        a1_s = mlp_pool.tile([M1, sl], BF16, tag="v1a")
        nc.scalar.activation(a1_s[:], v1_s[:], ACTF.Relu, bias=of1, scale=sc1)

        # ---------- layer 2 ----------
        v2_s = mlp_pool.tile([M2, sl], F32, tag="v2")
        sp2_sum = stat_pool.tile([M2, 4], F32, tag="sp2sum")
        sp2_sq = stat_pool.tile([M2, 4], F32, tag="sp2sq")
        for k, (a, b) in enumerate(col_tiles()):
            pv = ps_a.tile([128, 512], F32, tag="p1")
            nc.tensor.matmul(pv[0:M2, :b - a], mw2t_s, a1_s[:, a:b],
                             start=True, stop=True)
            nc.vector.tensor_copy(v2_s[:, a:b], pv[0:M2, :b - a])
            nc.vector.tensor_reduce(sp2_sum[:, k:k + 1], pv[0:M2, :b - a],
                                    axis=AX.X, op=OP.add)
            scr2 = mlp_pool.tile([M2, 512], F32, tag="scr2", bufs=2)
            nc.scalar.activation(scr2[:, :b - a], pv[0:M2, :b - a],
                                 ACTF.Square,
                                 accum_out=sp2_sq[:, k:k + 1])
        s2 = stat_pool.tile([M2, 2], F32, tag="s2")
        nc.vector.tensor_reduce(s2[:, 0:1], sp2_sum[:], axis=AX.X, op=OP.add)
        nc.vector.tensor_reduce(s2[:, 1:2], sp2_sq[:], axis=AX.X, op=OP.add)
        cc_in2 = dram.tile([M2, 2], F32)
        cc_out2 = dram.tile([M2, 2], F32)
        nc.sync.dma_start(cc_in2[:], s2[:])
        nc.gpsimd.collective_compute(
            "AllReduce", OP.add, replica_groups=[list(range(N_CORES))],
            ins=[cc_in2.opt()], outs=[cc_out2.opt()])

        # empty-slot correction, computed while the AllReduce is in flight:
        # a1_empty = relu(of1); ve = W2 @ a1_empty
        a1e = stat_pool.tile([M1, 1], BF16, tag="a1e")
        nc.scalar.activation(a1e[:], of1, ACTF.Relu)
        pve = ps_b.tile([128, 512], F32, tag="p2")
        nc.tensor.matmul(pve[0:M2, 0:1], mw2t_s, a1e[:], start=True, stop=True)
        ve = stat_pool.tile([M2, 2], F32, tag="ve")
        nc.vector.tensor_copy(ve[:, 0:1], pve[0:M2, 0:1])
        nc.vector.tensor_tensor(ve[:, 1:2], ve[:, 0:1], ve[:, 0:1], OP.mult)

        ar2 = stat_pool.tile([M2, 2], F32, tag="ar2")
        nc.sync.dma_start(ar2[:], cc_out2[:])
        s2c = stat_pool.tile([M2, 2], F32, tag="s2c")
        nc.vector.scalar_tensor_tensor(
            s2c[:, 0:1], ve[:, 0:1], -n_empty, ar2[:, 0:1], OP.mult, OP.add)
        nc.vector.scalar_tensor_tensor(
            s2c[:, 1:2], ve[:, 1:2], -n_empty, ar2[:, 1:2], OP.mult, OP.add)

        t2 = stat_pool.tile([M2, 8], F32, tag="t2")
        mean2, m22, var2, inv2, rstd2, sc2, ms2, of2 = (
            t2[:, i:i + 1] for i in range(8))
        nc.vector.tensor_scalar(mean2, s2c[:, 0:1], inv_b, None, OP.mult)
        nc.vector.tensor_tensor(m22, mean2, mean2, OP.mult)
        nc.vector.tensor_scalar(m22, m22, EPS, None, OP.subtract)
        nc.vector.scalar_tensor_tensor(
            var2, s2c[:, 1:2], inv_b, m22, OP.mult, OP.subtract)
        nc.vector.reciprocal(inv2, var2)
        nc.scalar.activation(rstd2, inv2, ACTF.Sqrt)
        nc.vector.tensor_tensor(sc2, g2_s, rstd2, OP.mult)
        nc.vector.tensor_tensor(ms2, mean2, sc2, OP.mult)
        nc.vector.tensor_tensor(of2, bb2_s, ms2, OP.subtract)

        a2_s = mlp_pool.tile([M2, sl], BF16, tag="v2a")
        nc.scalar.activation(a2_s[:], v2_s[:], ACTF.Relu, bias=of2, scale=sc2)

        y_s = mlp_pool.tile([1, sl], F32, tag="y")
        for a, b in col_tiles():
            pv = ps_a.tile([128, 512], F32, tag="p1")
            nc.tensor.matmul(pv[0:1, :b - a], mw3t_s, a2_s[:, a:b],
                             start=True, stop=True)
            nc.scalar.activation(y_s[:, a:b], pv[0:1, :b - a], ACTF.Sigmoid,
                                 bias=b3_s)
        nc.sync.dma_start(y_out[:], y_s[:])

    nc.compile()
    return nc


_CACHE = {}


def kernel(**inputs) -> np.ndarray:
    x = np.asarray(inputs["x_set"], np.float32)[0]        # [16, T]
    x_scalar = np.asarray(inputs["x_scalar"], np.float32)  # [B, 8]
    lengths = np.asarray(inputs["sample_indices"])[0].astype(np.int64)
    b_total = x_scalar.shape[0]
    offsets = np.concatenate([[0], np.cumsum(lengths)[:-1]])

    groups, ng = _plan_groups(lengths, b_total)
    sl = ng * SLOTS
    per_core = [
        _prep_core(x, x_scalar, lengths, offsets, groups[c], ng)
        for c in range(N_CORES)
    ]

    b1 = np.asarray(inputs["phi_b1"], np.float32)
    b2 = np.asarray(inputs["phi_b2"], np.float32)
    assert np.all(b1 == 0.0) and np.all(b2 == 0.0), \
        "nonzero phi bias path not implemented"

    cb = np.zeros((128, 449), dtype=np.float32)
    w1t = np.asarray(inputs["phi_w1"], np.float32).T      # [16, 64]
    w2t = np.asarray(inputs["phi_w2"], np.float32).T      # [64, 64]
    cb[0:F, 0:F] = w2t
    cb[F:128, F:128] = w2t
    cb[0:C_IN, 128:128 + F] = w1t
    cb[C_IN:2 * C_IN, 128 + F:256] = w1t
    cb[0:F + S_SCALAR, 256:384] = np.asarray(inputs["mlp_w1"], np.float32).T
    cb[0:M1, 384:448] = np.asarray(inputs["mlp_w2"], np.float32).T
    cb[0:M2, 448:449] = np.asarray(inputs["mlp_w3"], np.float32).T

    cf = np.zeros((128, 5), dtype=np.float32)
    cf[:, 0] = np.asarray(inputs["bn1_g"], np.float32)
    cf[:, 1] = np.asarray(inputs["bn1_b"], np.float32)
    cf[0:M2, 2] = np.asarray(inputs["bn2_g"], np.float32)
    cf[0:M2, 3] = np.asarray(inputs["bn2_b"], np.float32)
    cf[0, 4] = float(np.asarray(inputs["mlp_b3"], np.float32).reshape(()))

    consts = {
        "cb": np.ascontiguousarray(cb.astype(ml_dtypes.bfloat16)),
        "cf": np.ascontiguousarray(cf),
    }

    key = (ng, sl, b_total)
    if key not in _CACHE:
        _CACHE[key] = _build_nc(ng, sl, b_total)
    nc = _CACHE[key]

    in_maps = []
    for pc in per_core:
        m = {"xb": pc["xb"], "S": pc["S"], "xsT": pc["xsT"]}
        m.update(consts)
        in_maps.append(m)

    res = bass_utils.run_bass_kernel_spmd(
        nc, in_maps, core_ids=list(range(N_CORES)))

    y = np.zeros((b_total, 1), dtype=np.float32)
    for c, pc in enumerate(per_core):
        ys = res.results[c]["y"][0]
        se = pc["slot_events"]
        mask = se >= 0
        y[se[mask], 0] = ys[mask]
    return y
